# revision 1
# baseline (speedup 1.0000x reference)
"""ConvMod kernel for Trainium2 (8 NeuronCores, batch-parallel).

Per-sample modulated 3x3 grouped conv:
  style = w @ (fce_kernel*fce_scale) + fce_bias                [B, CIN]
  wp    = conv_kernel * conv_scale * style                     [B,3,3,CIN,NF]
  wpp   = wp * rsqrt(sum(wp^2, (ky,kx,cin)) + 1e-8)            demodulated
  out   = conv2d_same(x, wpp per-sample) + conv_bias           [B,H,W,NF]

Sharding: batch B=8 across 8 cores (1 sample/core), params replicated.
Device layout: channels on partitions; host transposes x to [C,H,W] per
sample and the [NF, H*W] device output back to NHWC.
"""

import numpy as np

B, H, W, CIN = 8, 256, 256, 64
WDIM, NF, KK = 512, 64, 3
NCORES = 8
CR = 32  # output rows per x chunk
FCE_SCALE = float(np.sqrt(1.0 / WDIM))
CONV_SCALE = float(np.sqrt(1.0 / 0.6 / (KK * KK * CIN)))

_CACHE = {}


def _build(repeats=1):
    import os as _os
    _skip_mm = _os.environ.get('KSKIP_MM') == '1'
    _skip_dup = _os.environ.get('KSKIP_DUP') == '1'
    import concourse.bass as bass_mod
    import concourse.mybir as mybir
    import concourse.tile as tile
    from concourse import bacc

    f32 = mybir.dt.float32
    f32r = mybir.dt.float32r
    nc = bacc.Bacc("TRN2", target_bir_lowering=False, debug=False,
                   num_devices=NCORES)

    xt = nc.dram_tensor("xt", [CIN, H, W], f32r, kind="ExternalInput").ap()
    wv = nc.dram_tensor("wv", [WDIM], f32, kind="ExternalInput").ap()
    fce_k = nc.dram_tensor("fce_k", [WDIM, CIN], f32, kind="ExternalInput").ap()
    fce_b = nc.dram_tensor("fce_b", [CIN], f32, kind="ExternalInput").ap()
    ck_d = nc.dram_tensor("ck", [KK, KK, CIN, NF], f32, kind="ExternalInput").ap()
    cb_d = nc.dram_tensor("cb", [NF], f32, kind="ExternalInput").ap()
    zpad = nc.dram_tensor("zpad", [CIN, W + 2], f32r, kind="ExternalInput").ap()
    yt = nc.dram_tensor("yt", [NF, H * W], f32, kind="ExternalOutput").ap()

    NT = KK * KK  # 9 taps
    with tile.TileContext(nc) as tc:
        with (
            tc.tile_pool(name="const", bufs=1) as const,
            tc.tile_pool(name="prep", bufs=1) as prep,
            tc.tile_pool(name="pps", bufs=1, space="PSUM") as pps,
            tc.tile_pool(name="xin", bufs=3) as xin,
            tc.tile_pool(name="yout", bufs=4) as yout,
            tc.tile_pool(name="acc", bufs=5, space="PSUM") as accp,
        ):
            # ---- weight prep (tiny) ----
            fce_sb = prep.tile([128, WDIM // 128, CIN], f32)
            nc.sync.dma_start(out=fce_sb,
                              in_=fce_k.rearrange("(j p) c -> p j c", p=128))
            wv_sb = prep.tile([128, WDIM // 128], f32)
            nc.sync.dma_start(out=wv_sb,
                              in_=wv.rearrange("(j p) -> p j", p=128))
            fce_b_sb = prep.tile([CIN, 1], f32)
            nc.sync.dma_start(out=fce_b_sb, in_=fce_b)
            ck_sb = prep.tile([CIN, NT, NF], f32)
            nc.sync.dma_start(out=ck_sb,
                              in_=ck_d.rearrange("ky kx c n -> c ky kx n"))
            cb_sb = const.tile([NF, 1], f32)
            nc.sync.dma_start(out=cb_sb, in_=cb_d)
            # conv_bias replicated on partitions 64-127 for col-tiled pairs
            cb2_sb = const.tile([2 * NF, 1], f32)
            nc.sync.dma_start(out=cb2_sb[0:NF, :], in_=cb_d)
            nc.sync.dma_start(out=cb2_sb[NF:2 * NF, :], in_=cb_d)

            ones_k = const.tile([CIN, 1], f32)
            nc.vector.memset(ones_k, 1.0)
            ones_m = const.tile([1, NF], f32)
            nc.vector.memset(ones_m, 1.0)

            # style = w @ (fce_k * fce_scale) + fce_b, then * conv_scale
            style_ps = pps.tile([CIN, 1], f32)
            for j in range(WDIM // 128):
                nc.tensor.matmul(style_ps, lhsT=fce_sb[:, j, :],
                                 rhs=wv_sb[:, j:j + 1],
                                 start=(j == 0), stop=(j == WDIM // 128 - 1))
            fce_b_sc = prep.tile([CIN, 1], f32)
            nc.scalar.mul(out=fce_b_sc, in_=fce_b_sb, mul=CONV_SCALE)
            stylec = prep.tile([CIN, 1], f32)
            nc.scalar.activation(stylec, style_ps,
                                 mybir.ActivationFunctionType.Identity,
                                 bias=fce_b_sc, scale=FCE_SCALE * CONV_SCALE)

            # wp[c, t, n] = ck * stylec[c];  sq = wp^2
            wp = prep.tile([CIN, NT, NF], f32)
            nc.vector.tensor_scalar_mul(wp.rearrange("c t n -> c (t n)"),
                                        ck_sb.rearrange("c t n -> c (t n)"),
                                        stylec)
            sq = prep.tile([CIN, NT, NF], f32)
            nc.vector.tensor_mul(sq.rearrange("c t n -> c (t n)"),
                                 wp.rearrange("c t n -> c (t n)"),
                                 wp.rearrange("c t n -> c (t n)"))
            tap_acc = prep.tile([CIN, NF], f32)
            nc.vector.tensor_add(tap_acc, sq[:, 0, :], sq[:, 1, :])
            for t in range(2, NT):
                nc.vector.tensor_add(tap_acc, tap_acc, sq[:, t, :])
            ssum_ps = pps.tile([1, NF], f32)
            nc.tensor.matmul(ssum_ps, lhsT=ones_k, rhs=tap_acc,
                             start=True, stop=True)
            eps_sb = prep.tile([1, 1], f32)
            nc.vector.memset(eps_sb, 1e-8)
            sroot = prep.tile([1, NF], f32)
            nc.scalar.activation(sroot, ssum_ps,
                                 mybir.ActivationFunctionType.Sqrt,
                                 bias=eps_sb, scale=1.0)
            wstd = prep.tile([1, NF], f32)
            nc.vector.reciprocal(wstd, sroot)
            bcast_ps = pps.tile([CIN, NF], f32)
            nc.tensor.matmul(bcast_ps, lhsT=ones_m, rhs=wstd,
                             start=True, stop=True)
            wsb = const.tile([CIN, NT, NF], f32r)
            for t in range(NT):
                nc.vector.tensor_mul(wsb[:, t, :], wp[:, t, :], bcast_ps)

            # stacked weights for K-packed tap pairs: partitions 0-63 hold
            # tap (dy,-1), partitions 64-127 hold tap (dy,0); a single K=128
            # matmul then contracts both taps at once (rhs band B = x
            # shifted left one column).
            w2 = const.tile([2 * CIN, KK, NF], f32r)
            for dyi in range(KK):
                nc.sync.dma_start(out=w2[0:CIN, dyi, :],
                                  in_=wsb[:, dyi * 3 + 0, :])
                nc.sync.dma_start(out=w2[CIN:2 * CIN, dyi, :],
                                  in_=wsb[:, dyi * 3 + 1, :])

            # ---- main conv loop ----
            NCH = H // CR           # chunks
            GPC = CR // 2           # row-pair groups per chunk
            for _ in range(repeats):
                for ci in range(NCH):
                    xc = xin.tile([2 * CIN, CR + 2, W + 2], f32r)
                    r0 = ci * CR - 1
                    l0 = 1 if ci == 0 else 0
                    l1 = (CR + 1) if ci == NCH - 1 else (CR + 2)
                    nc.sync.dma_start(
                        out=xc[0:CIN, l0:l1, 1:W + 1],
                        in_=xt[:, max(r0, 0):min(r0 + CR + 2, H), :])
                    nc.sync.dma_start(out=xc[0:CIN, :, 0:1],
                                      in_=zpad[:, 0:CR + 2])
                    nc.sync.dma_start(out=xc[0:CIN, :, W + 1:W + 2],
                                      in_=zpad[:, 0:CR + 2])
                    if ci == 0:
                        nc.sync.dma_start(out=xc[0:CIN, 0:1, :], in_=zpad)
                    if ci == NCH - 1:
                        nc.sync.dma_start(out=xc[0:CIN, CR + 1:CR + 2, :],
                                          in_=zpad)
                    # band B: same rows, shifted left one column
                    if not _skip_dup:
                        # split into row bands so early matmuls overlap the copy
                        nb = 4
                        rb = (CR + 2 + nb - 1) // nb
                        for bi in range(nb):
                            a, b = bi * rb, min((bi + 1) * rb, CR + 2)
                            if a >= b:
                                continue
                            nc.vector.tensor_copy(
                                xc[CIN:2 * CIN, a:b, 0:W + 1],
                                xc[0:CIN, a:b, 1:W + 2])

                    for gi in range(GPC):
                        g = ci * GPC + gi
                        s = g % 4
                        if s == 0:
                            ys = yout.tile([NF, 4, 2 * W], f32)
                        ps = accp.tile([NF, 2 * W], f32)
                        if not _skip_mm:
                            for dyi in range(KK):
                                lrow = gi * 2 + dyi
                                rhs = xc[:, lrow:lrow + 2, 0:W]
                                nc.tensor.matmul(ps, lhsT=w2[:, dyi, :], rhs=rhs,
                                                 start=(dyi == 0), stop=False)
                            for dyi in range(KK):
                                lrow = gi * 2 + dyi
                                rhs = xc[0:CIN, lrow:lrow + 2, 2:2 + W]
                                nc.tensor.matmul(ps, lhsT=wsb[:, dyi * 3 + 2, :],
                                                 rhs=rhs, start=False,
                                                 stop=(dyi == KK - 1))
                        else:
                            nc.tensor.matmul(ps, lhsT=wsb[:, 0, :],
                                             rhs=xc[0:CIN, 0:2, 0:W],
                                             start=True, stop=True)
                        nc.scalar.activation(ys[:, s, :], ps,
                                             mybir.ActivationFunctionType.Identity,
                                             bias=cb_sb, scale=1.0)
                        if s == 3:
                            nc.sync.dma_start(
                                out=yt[:, (g - 3) * 2 * W:(g + 1) * 2 * W],
                                in_=ys)

    nc.compile()
    return nc


def _get(repeats=1):
    if repeats not in _CACHE:
        _CACHE[repeats] = _build(repeats)
    return _CACHE[repeats]


def kernel(x, w, fce_kernel, fce_bias, conv_kernel, conv_bias):
    from concourse.bass_utils import run_bass_kernel_spmd

    nc = _get()
    in_maps = []
    for b in range(B):
        in_maps.append({
            "xt": np.ascontiguousarray(np.asarray(x[b], np.float32).transpose(2, 0, 1)),
            "wv": np.ascontiguousarray(np.asarray(w[b], np.float32)),
            "fce_k": np.asarray(fce_kernel, np.float32),
            "fce_b": np.asarray(fce_bias, np.float32),
            "ck": np.asarray(conv_kernel, np.float32),
            "cb": np.asarray(conv_bias, np.float32),
            "zpad": np.zeros((CIN, W + 2), np.float32),
        })
    res = run_bass_kernel_spmd(nc, in_maps, core_ids=list(range(NCORES)))
    out = np.empty((B, H, W, NF), np.float32)
    for b in range(B):
        out[b] = res.results[b]["yt"].reshape(NF, H, W).transpose(1, 2, 0)
    return out



# revision 4
# speedup vs baseline: 2.2817x; 2.2817x over previous
"""ConvMod kernel for Trainium2 (8 NeuronCores, batch-parallel).

Per-sample modulated 3x3 grouped conv:
  style = w @ (fce_kernel*fce_scale) + fce_bias                [B, CIN]
  wp    = conv_kernel * conv_scale * style                     [B,3,3,CIN,NF]
  wpp   = wp * rsqrt(sum(wp^2, (ky,kx,cin)) + 1e-8)            demodulated
  out   = conv2d_same(x, wpp per-sample) + conv_bias           [B,H,W,NF]

Sharding: batch B=8 across 8 cores (1 sample/core), params replicated.

Device conv scheme (column-parity packing, bf16):
  - x is padded+cast to bf16 on host: xb[c, t, u] = x[t-1, u-1, c]
    ([CIN, H+2, W+2], zeros outside).
  - SBUF tile xc [128, 34, 258]: partitions 0-63 = chunk window of xb,
    partitions 64-127 = same window shifted left 2 columns.
  - One matmul computes BOTH column parities: out partition m=(s,n) with
    s = output-column parity, lhsT [128, 2, NF]; rhs reads columns with
    stride 2.  Slot (band, s) of matmul (ar, ac) carries conv tap
    ky=ar, kx=ac+2*band-s (weights zeroed for kx outside 0..2).
  - 6 matmuls (ar in 0..2, ac in 0..1) per 4-output-row strip accumulate
    the full 3x3x64 contraction into PSUM [128, 4, 128].
  - ACT evacuates PSUM -> bf16 SBUF with conv_bias, DMA to yt
    [128, H*W/2] (parity-interleaved); host de-interleaves.
"""

import numpy as np
import ml_dtypes

B, H, W, CIN = 8, 256, 256, 64
WDIM, NF, KK = 512, 64, 3
NCORES = 8
CR = 32             # output rows per chunk
NCH = H // CR       # chunks
SPC = CR // 4       # 4-row strips per chunk
FCE_SCALE = float(np.sqrt(1.0 / WDIM))
CONV_SCALE = float(np.sqrt(1.0 / 0.6 / (KK * KK * CIN)))

_CACHE = {}


def _build(repeats=1):
    import concourse.bass as bass_mod
    import concourse.mybir as mybir
    import concourse.tile as tile
    from concourse import bacc

    f32 = mybir.dt.float32
    bf16 = mybir.dt.bfloat16
    nc = bacc.Bacc("TRN2", target_bir_lowering=False, debug=False,
                   num_devices=NCORES)

    xbd = nc.dram_tensor("xb", [CIN, H + 2, W + 2], bf16,
                         kind="ExternalInput").ap()
    wv = nc.dram_tensor("wv", [WDIM], f32, kind="ExternalInput").ap()
    fce_k = nc.dram_tensor("fce_k", [WDIM, CIN], f32, kind="ExternalInput").ap()
    fce_b = nc.dram_tensor("fce_b", [CIN], f32, kind="ExternalInput").ap()
    ck_d = nc.dram_tensor("ck", [KK, KK, CIN, NF], f32, kind="ExternalInput").ap()
    cb_d = nc.dram_tensor("cb", [NF], f32, kind="ExternalInput").ap()
    yt = nc.dram_tensor("yt", [2 * NF, H * W // 2], bf16,
                        kind="ExternalOutput").ap()

    NT = KK * KK  # 9 taps
    with tile.TileContext(nc) as tc:
        with (
            tc.tile_pool(name="const", bufs=1) as const,
            tc.tile_pool(name="prep", bufs=1) as prep,
            tc.tile_pool(name="pps", bufs=1, space="PSUM") as pps,
            tc.tile_pool(name="xin", bufs=3) as xin,
            tc.tile_pool(name="yout", bufs=4) as yout,
            tc.tile_pool(name="acc", bufs=5, space="PSUM") as accp,
        ):
            # ---- weight prep (tiny, fp32) ----
            fce_sb = prep.tile([128, WDIM // 128, CIN], f32)
            nc.sync.dma_start(out=fce_sb,
                              in_=fce_k.rearrange("(j p) c -> p j c", p=128))
            wv_sb = prep.tile([128, WDIM // 128], f32)
            nc.sync.dma_start(out=wv_sb,
                              in_=wv.rearrange("(j p) -> p j", p=128))
            fce_b_sb = prep.tile([CIN, 1], f32)
            nc.sync.dma_start(out=fce_b_sb, in_=fce_b)
            ck_sb = prep.tile([CIN, NT, NF], f32)
            nc.sync.dma_start(out=ck_sb,
                              in_=ck_d.rearrange("ky kx c n -> c ky kx n"))
            # conv_bias replicated on partitions 64-127 for the parity pair
            cb2_sb = const.tile([2 * NF, 1], f32)
            nc.sync.dma_start(out=cb2_sb[0:NF, :], in_=cb_d)
            nc.sync.dma_start(out=cb2_sb[NF:2 * NF, :], in_=cb_d)

            ones_k = const.tile([CIN, 1], f32)
            nc.vector.memset(ones_k, 1.0)
            ones_m = const.tile([1, NF], f32)
            nc.vector.memset(ones_m, 1.0)

            # style = w @ (fce_k * fce_scale) + fce_b, then * conv_scale
            style_ps = pps.tile([CIN, 1], f32)
            for j in range(WDIM // 128):
                nc.tensor.matmul(style_ps, lhsT=fce_sb[:, j, :],
                                 rhs=wv_sb[:, j:j + 1],
                                 start=(j == 0), stop=(j == WDIM // 128 - 1))
            fce_b_sc = prep.tile([CIN, 1], f32)
            nc.scalar.mul(out=fce_b_sc, in_=fce_b_sb, mul=CONV_SCALE)
            stylec = prep.tile([CIN, 1], f32)
            nc.scalar.activation(stylec, style_ps,
                                 mybir.ActivationFunctionType.Identity,
                                 bias=fce_b_sc, scale=FCE_SCALE * CONV_SCALE)

            # wp[c, t, n] = ck * stylec[c];  sq = wp^2
            wp = prep.tile([CIN, NT, NF], f32)
            nc.vector.tensor_scalar_mul(wp.rearrange("c t n -> c (t n)"),
                                        ck_sb.rearrange("c t n -> c (t n)"),
                                        stylec)
            sq = prep.tile([CIN, NT, NF], f32)
            nc.vector.tensor_mul(sq.rearrange("c t n -> c (t n)"),
                                 wp.rearrange("c t n -> c (t n)"),
                                 wp.rearrange("c t n -> c (t n)"))
            tap_acc = prep.tile([CIN, NF], f32)
            nc.vector.tensor_add(tap_acc, sq[:, 0, :], sq[:, 1, :])
            for t in range(2, NT):
                nc.vector.tensor_add(tap_acc, tap_acc, sq[:, t, :])
            ssum_ps = pps.tile([1, NF], f32)
            nc.tensor.matmul(ssum_ps, lhsT=ones_k, rhs=tap_acc,
                             start=True, stop=True)
            eps_sb = prep.tile([1, 1], f32)
            nc.vector.memset(eps_sb, 1e-8)
            sroot = prep.tile([1, NF], f32)
            nc.scalar.activation(sroot, ssum_ps,
                                 mybir.ActivationFunctionType.Sqrt,
                                 bias=eps_sb, scale=1.0)
            wstd = prep.tile([1, NF], f32)
            nc.vector.reciprocal(wstd, sroot)
            bcast_ps = pps.tile([CIN, NF], f32)
            nc.tensor.matmul(bcast_ps, lhsT=ones_m, rhs=wstd,
                             start=True, stop=True)
            wsb = prep.tile([CIN, NT, NF], f32)
            for t in range(NT):
                nc.vector.tensor_mul(wsb[:, t, :], wp[:, t, :], bcast_ps)

            # parity-packed weight tiles: wt[ar][ac] is lhsT [128, 2, NF]
            # (partition k=(band,ci), free m=(s,n)); slot (band, s) holds
            # conv tap ky=ar, kx=ac+2*band-s, zero when kx outside 0..2.
            wt_all = const.tile([2 * CIN, KK * 2, 2, NF], bf16)
            for ar in range(KK):
                for ac in range(2):
                    for band in range(2):
                        for s in range(2):
                            kx = ac + 2 * band - s
                            dst = wt_all[band * CIN:(band + 1) * CIN,
                                         ar * 2 + ac, s, :]
                            if 0 <= kx < KK:
                                nc.vector.tensor_copy(dst,
                                                      wsb[:, ar * KK + kx, :])
                            else:
                                nc.vector.memset(dst, 0.0)

            # ---- main conv loop ----
            for _ in range(repeats):
                for ci in range(NCH):
                    L = ci * CR
                    xc = xin.tile([2 * CIN, CR + 2, W + 2], bf16)
                    nc.sync.dma_start(out=xc[0:CIN, :, :],
                                      in_=xbd[:, L:L + CR + 2, :])
                    # band B: same rows, shifted left two columns; split so
                    # early strips can start before the whole copy is done
                    for (a, b) in ((0, 12), (12, 24), (24, CR + 2)):
                        nc.vector.tensor_copy(
                            xc[CIN:2 * CIN, a:b, 0:W],
                            xc[0:CIN, a:b, 2:W + 2])
                    for gi in range(SPC):
                        g = ci * SPC + gi
                        ps = accp.tile([2 * NF, 4, W // 2], f32)
                        k = 0
                        for ar in range(KK):
                            for ac in range(2):
                                rhs = xc[:, 4 * gi + ar:4 * gi + ar + 4,
                                         ac:ac + 2 * (W // 2):2]
                                nc.tensor.matmul(ps,
                                                 lhsT=wt_all[:, ar * 2 + ac],
                                                 rhs=rhs,
                                                 start=(k == 0), stop=(k == 5))
                                k += 1
                        ys = yout.tile([2 * NF, 4, W // 2], bf16)
                        nc.scalar.activation(ys, ps,
                                             mybir.ActivationFunctionType.Identity,
                                             bias=cb2_sb, scale=1.0)
                        nc.sync.dma_start(
                            out=yt[:, g * 2 * W:(g + 1) * 2 * W],
                            in_=ys)

    nc.compile()
    return nc


def _get(repeats=1):
    if repeats not in _CACHE:
        _CACHE[repeats] = _build(repeats)
    return _CACHE[repeats]


def _host_pack_x(x_b):
    """[H, W, CIN] fp32 -> padded bf16 [CIN, H+2, W+2]."""
    xb = np.zeros((CIN, H + 2, W + 2), dtype=ml_dtypes.bfloat16)
    xb[:, 1:H + 1, 1:W + 1] = np.ascontiguousarray(
        x_b.transpose(2, 0, 1)).astype(ml_dtypes.bfloat16)
    return xb


def _in_maps(x, w, fce_kernel, fce_bias, conv_kernel, conv_bias):
    maps = []
    for b in range(B):
        maps.append({
            "xb": _host_pack_x(np.asarray(x[b], np.float32)),
            "wv": np.ascontiguousarray(np.asarray(w[b], np.float32)),
            "fce_k": np.asarray(fce_kernel, np.float32),
            "fce_b": np.asarray(fce_bias, np.float32),
            "ck": np.asarray(conv_kernel, np.float32),
            "cb": np.asarray(conv_bias, np.float32),
        })
    return maps


def kernel(x, w, fce_kernel, fce_bias, conv_kernel, conv_bias):
    from concourse.bass_utils import run_bass_kernel_spmd

    nc = _get()
    in_maps = _in_maps(x, w, fce_kernel, fce_bias, conv_kernel, conv_bias)
    res = run_bass_kernel_spmd(nc, in_maps, core_ids=list(range(NCORES)))
    out = np.empty((B, H, W, NF), np.float32)
    for b in range(B):
        yt = np.asarray(res.results[b]["yt"])         # [128, H*W/2] bf16
        # yt[s*NF+n, r*(W/2)+j] = y[r, 2j+s, n]
        y = yt.reshape(2, NF, H, W // 2).transpose(2, 3, 0, 1)
        out[b] = y.reshape(H, W, NF).astype(np.float32)
    return out


# revision 8
# speedup vs baseline: 2.3846x; 1.0451x over previous
"""ConvMod kernel for Trainium2 (8 NeuronCores, batch-parallel).

Per-sample modulated 3x3 grouped conv:
  style = w @ (fce_kernel*fce_scale) + fce_bias                [B, CIN]
  wp    = conv_kernel * conv_scale * style                     [B,3,3,CIN,NF]
  wpp   = wp * rsqrt(sum(wp^2, (ky,kx,cin)) + 1e-8)            demodulated
  out   = conv2d_same(x, wpp per-sample) + conv_bias           [B,H,W,NF]

Sharding: batch B=8 across 8 cores (1 sample/core), params replicated.

Device conv scheme (column-parity packing, bf16):
  - x is padded+cast to bf16 on host: xb[c, t, u] = x[t-1, u-1, c]
    ([CIN, H+2, W+2], zeros outside).
  - SBUF tile xc [128, 34, 258]: partitions 0-63 = chunk window of xb,
    partitions 64-127 = same window shifted left 2 columns.
  - One matmul computes BOTH column parities: out partition m=(s,n) with
    s = output-column parity, lhsT [128, 2, NF]; rhs reads columns with
    stride 2.  Slot (band, s) of matmul (ar, ac) carries conv tap
    ky=ar, kx=ac+2*band-s (weights zeroed for kx outside 0..2).
  - 6 matmuls (ar in 0..2, ac in 0..1) per 4-output-row strip accumulate
    the full 3x3x64 contraction into PSUM [128, 4, 128].
  - ACT evacuates PSUM -> bf16 SBUF with conv_bias, DMA to yt
    [128, H*W/2] (parity-interleaved); host de-interleaves.
"""

import numpy as np
import ml_dtypes

B, H, W, CIN = 8, 256, 256, 64
WDIM, NF, KK = 512, 64, 3
NCORES = 8
CR = 32             # output rows per chunk
NCH = H // CR       # chunks
SPC = CR // 4       # 4-row strips per chunk
FCE_SCALE = float(np.sqrt(1.0 / WDIM))
CONV_SCALE = float(np.sqrt(1.0 / 0.6 / (KK * KK * CIN)))

_CACHE = {}


def _build(repeats=1):
    import concourse.bass as bass_mod
    import concourse.mybir as mybir
    import concourse.tile as tile
    from concourse import bacc

    f32 = mybir.dt.float32
    bf16 = mybir.dt.bfloat16
    nc = bacc.Bacc("TRN2", target_bir_lowering=False, debug=False,
                   num_devices=NCORES)

    xbd = nc.dram_tensor("xb", [CIN, H + 2, W + 2], bf16,
                         kind="ExternalInput").ap()
    # host-packed params (fewer DMAs, scales pre-applied):
    # fkw [128, 4*65+1]: per j: 64 cols scaled fce_k + 1 col wv; last col
    # is conv_bias replicated for both parities.
    fkw_d = nc.dram_tensor("fkw", [128, (WDIM // 128) * (CIN + 1) + 1], f32,
                           kind="ExternalInput").ap()
    # ckb [64, 577]: conv_kernel as [c, (ky kx n)] + scaled fce_bias col.
    ckb_d = nc.dram_tensor("ckb", [CIN, KK * KK * NF + 1], f32,
                           kind="ExternalInput").ap()
    yt = nc.dram_tensor("yt", [2 * NF, H * W // 2], bf16,
                        kind="ExternalOutput").ap()

    NT = KK * KK  # 9 taps
    with tile.TileContext(nc) as tc:
        with (
            tc.tile_pool(name="const", bufs=1) as const,
            tc.tile_pool(name="prep", bufs=1) as prep,
            tc.tile_pool(name="pps", bufs=1, space="PSUM") as pps,
            tc.tile_pool(name="xin", bufs=3) as xin,
            tc.tile_pool(name="yout", bufs=4) as yout,
            tc.tile_pool(name="acc", bufs=5, space="PSUM") as accp,
        ):
            # ---- weight prep (tiny, fp32) ----
            JW = WDIM // 128
            fkw_sb = prep.tile([128, JW * (CIN + 1) + 1], f32)
            nc.sync.dma_start(out=fkw_sb, in_=fkw_d)
            ckb_sb = prep.tile([CIN, NT * NF + 1], f32)
            nc.sync.dma_start(out=ckb_sb, in_=ckb_d)
            cb2_sb = fkw_sb[:, JW * (CIN + 1):JW * (CIN + 1) + 1]

            ones_k = const.tile([CIN, 1], f32)
            nc.vector.memset(ones_k, 1.0)
            ones_m = const.tile([1, NF], f32)
            nc.vector.memset(ones_m, 1.0)
            eps_sb = prep.tile([1, 1], f32)
            nc.vector.memset(eps_sb, 1e-8)
            # parity-packed weight tiles, zero slots memset up front:
            # wt_all[:, ar*2+ac] is lhsT [128, 2, NF]; slot (band, s) holds
            # conv tap ky=ar, kx=ac+2*band-s, zero when kx outside 0..2.
            wt_all = const.tile([2 * CIN, KK * 2, 2, NF], bf16)
            for ar in range(KK):
                nc.vector.memset(wt_all[0:CIN, ar * 2, 1, :], 0.0)
                nc.vector.memset(wt_all[CIN:2 * CIN, ar * 2 + 1, 0, :], 0.0)

            # style = w @ (fce_k * fce_scale * conv_scale)  (scales host-baked)
            style_ps = pps.tile([CIN, 1], f32)
            for j in range(JW):
                base = j * (CIN + 1)
                nc.tensor.matmul(style_ps, lhsT=fkw_sb[:, base:base + CIN],
                                 rhs=fkw_sb[:, base + CIN:base + CIN + 1],
                                 start=(j == 0), stop=(j == JW - 1))
            stylec = prep.tile([CIN, 1], f32)
            nc.vector.tensor_scalar_add(stylec, style_ps,
                                        ckb_sb[:, NT * NF:NT * NF + 1])

            # wp[c, t, n] = ck * stylec[c];  sq = wp^2; demod
            wp = prep.tile([CIN, NT, NF], f32)
            nc.vector.tensor_scalar_mul(wp.rearrange("c t n -> c (t n)"),
                                        ckb_sb[:, 0:NT * NF],
                                        stylec)
            sq = prep.tile([CIN, NT, NF], f32)
            nc.vector.tensor_mul(sq.rearrange("c t n -> c (t n)"),
                                 wp.rearrange("c t n -> c (t n)"),
                                 wp.rearrange("c t n -> c (t n)"))
            tap_acc = prep.tile([CIN, NF], f32)
            nc.vector.tensor_reduce(tap_acc, sq.rearrange("c t n -> c n t"),
                                    mybir.AxisListType.X, mybir.AluOpType.add)
            ssum_ps = pps.tile([1, NF], f32)
            nc.tensor.matmul(ssum_ps, lhsT=ones_k, rhs=tap_acc,
                             start=True, stop=True)
            sroot = prep.tile([1, NF], f32)
            nc.scalar.activation(sroot, ssum_ps,
                                 mybir.ActivationFunctionType.Sqrt,
                                 bias=eps_sb, scale=1.0)
            wstd = prep.tile([1, NF], f32)
            nc.vector.reciprocal(wstd, sroot)
            bcast_ps = pps.tile([CIN, NF], f32)
            nc.tensor.matmul(bcast_ps, lhsT=ones_m, rhs=wstd,
                             start=True, stop=True)
            # fused demod + cast into the parity-packed slots, ar-major so
            # the first strips' weights are ready earliest
            for ar in range(KK):
                for ac in range(2):
                    for band in range(2):
                        for s in range(2):
                            kx = ac + 2 * band - s
                            if 0 <= kx < KK:
                                dst = wt_all[band * CIN:(band + 1) * CIN,
                                             ar * 2 + ac, s, :]
                                nc.vector.tensor_mul(dst,
                                                     wp[:, ar * KK + kx, :],
                                                     bcast_ps)

            # ---- main conv loop ----
            for _ in range(repeats):
                for ci in range(NCH):
                    L = ci * CR
                    xc = xin.tile([2 * CIN, CR + 2, W + 2], bf16)
                    if ci == 0:
                        nc.sync.dma_start(out=xc[0:CIN, 0:14, :],
                                          in_=xbd[:, L:L + 14, :])
                        nc.sync.dma_start(out=xc[0:CIN, 14:CR + 2, :],
                                          in_=xbd[:, L + 14:L + CR + 2, :])
                    else:
                        nc.sync.dma_start(out=xc[0:CIN, :, :],
                                          in_=xbd[:, L:L + CR + 2, :])
                    # band B: same rows, shifted left two columns; split so
                    # early strips can start before the whole copy is done
                    for (a, b) in ((0, 12), (12, 24), (24, CR + 2)):
                        nc.vector.tensor_copy(
                            xc[CIN:2 * CIN, a:b, 0:W],
                            xc[0:CIN, a:b, 2:W + 2])
                    for gi in range(SPC):
                        g = ci * SPC + gi
                        ps = accp.tile([2 * NF, 4, W // 2], f32)
                        k = 0
                        for ar in range(KK):
                            for ac in range(2):
                                rhs = xc[:, 4 * gi + ar:4 * gi + ar + 4,
                                         ac:ac + 2 * (W // 2):2]
                                nc.tensor.matmul(ps,
                                                 lhsT=wt_all[:, ar * 2 + ac],
                                                 rhs=rhs,
                                                 start=(k == 0), stop=(k == 5))
                                k += 1
                        ys = yout.tile([2 * NF, 4, W // 2], bf16)
                        nc.scalar.activation(ys, ps,
                                             mybir.ActivationFunctionType.Identity,
                                             bias=cb2_sb, scale=1.0)
                        nc.sync.dma_start(
                            out=yt[:, g * 2 * W:(g + 1) * 2 * W],
                            in_=ys)

    nc.compile()
    return nc


def _get(repeats=1):
    if repeats not in _CACHE:
        _CACHE[repeats] = _build(repeats)
    return _CACHE[repeats]


def _host_pack_x(x_b):
    """[H, W, CIN] fp32 -> padded bf16 [CIN, H+2, W+2]."""
    xb = np.zeros((CIN, H + 2, W + 2), dtype=ml_dtypes.bfloat16)
    xb[:, 1:H + 1, 1:W + 1] = np.ascontiguousarray(
        x_b.transpose(2, 0, 1)).astype(ml_dtypes.bfloat16)
    return xb


def _in_maps(x, w, fce_kernel, fce_bias, conv_kernel, conv_bias):
    JW = WDIM // 128
    fce_k = np.asarray(fce_kernel, np.float32)
    # fce_k row index = j*128 + p  ->  [p, j, c], scales baked in
    fkc = fce_k.reshape(JW, 128, CIN).transpose(1, 0, 2) * np.float32(
        FCE_SCALE * CONV_SCALE)
    cb = np.asarray(conv_bias, np.float32)
    ckb = np.empty((CIN, KK * KK * NF + 1), np.float32)
    # ck [ky, kx, c, n] -> [c, (ky kx n)]
    ckb[:, :KK * KK * NF] = np.asarray(conv_kernel, np.float32).transpose(
        2, 0, 1, 3).reshape(CIN, KK * KK * NF)
    ckb[:, KK * KK * NF] = np.asarray(fce_bias, np.float32) * np.float32(
        CONV_SCALE)
    maps = []
    for b in range(B):
        wvb = np.asarray(w[b], np.float32).reshape(JW, 128).T  # [p, j]
        fkw = np.empty((128, JW * (CIN + 1) + 1), np.float32)
        for j in range(JW):
            fkw[:, j * (CIN + 1):j * (CIN + 1) + CIN] = fkc[:, j, :]
            fkw[:, j * (CIN + 1) + CIN] = wvb[:, j]
        fkw[:, JW * (CIN + 1)] = np.concatenate([cb, cb])
        maps.append({
            "xb": _host_pack_x(np.asarray(x[b], np.float32)),
            "fkw": fkw,
            "ckb": ckb,
        })
    return maps


def kernel(x, w, fce_kernel, fce_bias, conv_kernel, conv_bias):
    from concourse.bass_utils import run_bass_kernel_spmd

    nc = _get()
    in_maps = _in_maps(x, w, fce_kernel, fce_bias, conv_kernel, conv_bias)
    res = run_bass_kernel_spmd(nc, in_maps, core_ids=list(range(NCORES)))
    out = np.empty((B, H, W, NF), np.float32)
    for b in range(B):
        yt = np.asarray(res.results[b]["yt"])         # [128, H*W/2] bf16
        # yt[s*NF+n, r*(W/2)+j] = y[r, 2j+s, n]
        y = yt.reshape(2, NF, H, W // 2).transpose(2, 3, 0, 1)
        out[b] = y.reshape(H, W, NF).astype(np.float32)
    return out


# revision 10
# speedup vs baseline: 2.4093x; 1.0104x over previous
"""ConvMod kernel for Trainium2 (8 NeuronCores, batch-parallel).

Per-sample modulated 3x3 grouped conv:
  style = w @ (fce_kernel*fce_scale) + fce_bias                [B, CIN]
  wp    = conv_kernel * conv_scale * style                     [B,3,3,CIN,NF]
  wpp   = wp * rsqrt(sum(wp^2, (ky,kx,cin)) + 1e-8)            demodulated
  out   = conv2d_same(x, wpp per-sample) + conv_bias           [B,H,W,NF]

Sharding: batch B=8 across 8 cores (1 sample/core), params replicated.

Device conv scheme (column-parity packing, bf16):
  - x is padded+cast to bf16 on host: xb[c, t, u] = x[t-1, u-1, c]
    ([CIN, H+2, W+2], zeros outside).
  - SBUF tile xc [128, 34, 258]: partitions 0-63 = chunk window of xb,
    partitions 64-127 = same window shifted left 2 columns.
  - One matmul computes BOTH column parities: out partition m=(s,n) with
    s = output-column parity, lhsT [128, 2, NF]; rhs reads columns with
    stride 2.  Slot (band, s) of matmul (ar, ac) carries conv tap
    ky=ar, kx=ac+2*band-s (weights zeroed for kx outside 0..2).
  - 6 matmuls (ar in 0..2, ac in 0..1) per 4-output-row strip accumulate
    the full 3x3x64 contraction into PSUM [128, 4, 128].
  - ACT evacuates PSUM -> bf16 SBUF with conv_bias, DMA to yt
    [128, H*W/2] (parity-interleaved); host de-interleaves.
"""

import numpy as np
import ml_dtypes

B, H, W, CIN = 8, 256, 256, 64
WDIM, NF, KK = 512, 64, 3
NCORES = 8
CR = 32             # output rows per chunk
NCH = H // CR       # chunks
SPC = CR // 4       # 4-row strips per chunk
FCE_SCALE = float(np.sqrt(1.0 / WDIM))
CONV_SCALE = float(np.sqrt(1.0 / 0.6 / (KK * KK * CIN)))

_CACHE = {}


def _build(repeats=1):
    import concourse.bass as bass_mod
    import concourse.mybir as mybir
    import concourse.tile as tile
    from concourse import bacc

    f32 = mybir.dt.float32
    bf16 = mybir.dt.bfloat16
    nc = bacc.Bacc("TRN2", target_bir_lowering=False, debug=False,
                   num_devices=NCORES)

    xbd = nc.dram_tensor("xb", [CIN, H + 2, W + 2], bf16,
                         kind="ExternalInput").ap()
    # host-packed params (fewer DMAs, scales pre-applied):
    # fkw [128, 4*65+1]: per j: 64 cols scaled fce_k + 1 col wv; last col
    # is conv_bias replicated for both parities.
    fkw_d = nc.dram_tensor("fkw", [128, (WDIM // 128) * (CIN + 1) + 1], f32,
                           kind="ExternalInput").ap()
    # ckb [64, 577]: conv_kernel as [c, (ky kx n)] + scaled fce_bias col.
    ckb_d = nc.dram_tensor("ckb", [CIN, KK * KK * NF + 1], f32,
                           kind="ExternalInput").ap()
    yt = nc.dram_tensor("yt", [2 * NF, H * W // 2], bf16,
                        kind="ExternalOutput").ap()

    NT = KK * KK  # 9 taps
    with tile.TileContext(nc) as tc:
        with (
            tc.tile_pool(name="const", bufs=1) as const,
            tc.tile_pool(name="prep", bufs=1) as prep,
            tc.tile_pool(name="pps", bufs=1, space="PSUM") as pps,
            tc.tile_pool(name="xin", bufs=3) as xin,
            tc.tile_pool(name="yout", bufs=4) as yout,
            tc.tile_pool(name="acc", bufs=5, space="PSUM") as accp,
        ):
            # ---- weight prep (tiny, fp32) ----
            JW = WDIM // 128
            ckb_sb = prep.tile([CIN, NT * NF + 1], f32)
            nc.sync.dma_start(out=ckb_sb, in_=ckb_d)
            fkw_sb = prep.tile([128, JW * (CIN + 1) + 1], f32)
            nc.sync.dma_start(out=fkw_sb, in_=fkw_d)
            cb2_sb = fkw_sb[:, JW * (CIN + 1):JW * (CIN + 1) + 1]

            ones_k = const.tile([CIN, 1], f32)
            nc.vector.memset(ones_k, 1.0)
            one1 = const.tile([1, 1], f32)
            nc.vector.memset(one1, 1.0)
            eps_sb = prep.tile([1, 1], f32)
            nc.vector.memset(eps_sb, 1e-8)
            # Style-modulated (UN-demodulated) weights, parity-packed; the
            # demod rsqrt is applied per output channel at evacuation time
            # via the ACT scale operand.  wt[:, ar, jj, n]: the matmul for
            # (ar, ac) uses lhsT = wt[:, ar, 1-ac : 3-ac, :] so slot
            # (band, s) at jj = 1-ac+s holds tap ky=ar, kx=ac+2*band-s:
            #   band0 (parts 0-63):   jj0=kx1, jj1=kx0, jj2=zero
            #   band1 (parts 64-127): jj0=zero, jj1=kx2, jj2=kx1
            wt = const.tile([2 * CIN, KK, KK, NF], bf16)
            for ar in range(KK):
                nc.vector.memset(wt[0:CIN, ar, 2, :], 0.0)
                nc.vector.memset(wt[CIN:2 * CIN, ar, 0, :], 0.0)

            # style = w @ (fce_k * fce_scale * conv_scale)  (scales host-baked)
            style_ps = pps.tile([CIN, 1], f32)
            for j in range(JW):
                base = j * (CIN + 1)
                nc.tensor.matmul(style_ps, lhsT=fkw_sb[:, base:base + CIN],
                                 rhs=fkw_sb[:, base + CIN:base + CIN + 1],
                                 start=(j == 0), stop=(j == JW - 1))
            stylec = prep.tile([CIN, 1], f32)
            nc.vector.tensor_scalar_add(stylec, style_ps,
                                        ckb_sb[:, NT * NF:NT * NF + 1])

            # wp[c, t, n] = ck * stylec[c], then cast taps into wt slots
            wp = prep.tile([CIN, NT, NF], f32)
            nc.vector.tensor_scalar_mul(wp.rearrange("c t n -> c (t n)"),
                                        ckb_sb[:, 0:NT * NF],
                                        stylec)
            wp3 = wp.rearrange("c (t k) n -> c t k n", k=KK)
            nc.vector.tensor_copy(wt[0:CIN, :, 0, :], wp3[:, :, 1, :])
            nc.vector.tensor_copy(wt[0:CIN, :, 1, :], wp3[:, :, 0, :])
            nc.vector.tensor_copy(wt[CIN:2 * CIN, :, 1, :], wp3[:, :, 2, :])
            nc.vector.tensor_copy(wt[CIN:2 * CIN, :, 2, :], wp3[:, :, 1, :])

            # demod: wstd2[(s n), 1] = rsqrt(sum(wp^2) + 1e-8) per channel,
            # replicated for both parities; consumed by the evac ACT scale.
            sq = prep.tile([CIN, NT, NF], f32)
            nc.vector.tensor_mul(sq.rearrange("c t n -> c (t n)"),
                                 wp.rearrange("c t n -> c (t n)"),
                                 wp.rearrange("c t n -> c (t n)"))
            tap_acc = prep.tile([CIN, NF], f32)
            nc.vector.tensor_reduce(tap_acc, sq.rearrange("c t n -> c n t"),
                                    mybir.AxisListType.X, mybir.AluOpType.add)
            ssum_ps = pps.tile([1, NF], f32)
            nc.tensor.matmul(ssum_ps, lhsT=ones_k, rhs=tap_acc,
                             start=True, stop=True)
            sroot = prep.tile([1, NF], f32)
            nc.scalar.activation(sroot, ssum_ps,
                                 mybir.ActivationFunctionType.Sqrt,
                                 bias=eps_sb, scale=1.0)
            wstd = prep.tile([1, NF], f32)
            nc.vector.reciprocal(wstd, sroot)
            wstd2_ps = pps.tile([NF, 1], f32)
            nc.tensor.matmul(wstd2_ps, lhsT=wstd, rhs=one1,
                             start=True, stop=True)
            wstd2_sb = const.tile([2 * NF, 1], f32)
            nc.vector.tensor_copy(wstd2_sb[0:NF, :], wstd2_ps)
            nc.vector.tensor_copy(wstd2_sb[NF:2 * NF, :], wstd2_ps)

            # ---- main conv loop ----
            for _ in range(repeats):
                for ci in range(NCH):
                    L = ci * CR
                    xc = xin.tile([2 * CIN, CR + 2, W + 2], bf16)
                    if ci == 0:
                        nc.sync.dma_start(out=xc[0:CIN, 0:14, :],
                                          in_=xbd[:, L:L + 14, :])
                        nc.sync.dma_start(out=xc[0:CIN, 14:CR + 2, :],
                                          in_=xbd[:, L + 14:L + CR + 2, :])
                    else:
                        nc.sync.dma_start(out=xc[0:CIN, :, :],
                                          in_=xbd[:, L:L + CR + 2, :])
                    # band B: same rows, shifted left two columns; split so
                    # early strips can start before the whole copy is done
                    for (a, b) in ((0, 12), (12, 24), (24, CR + 2)):
                        nc.vector.tensor_copy(
                            xc[CIN:2 * CIN, a:b, 0:W],
                            xc[0:CIN, a:b, 2:W + 2])
                    for gi in range(SPC):
                        g = ci * SPC + gi
                        ps = accp.tile([2 * NF, 4, W // 2], f32)
                        k = 0
                        for ar in range(KK):
                            for ac in range(2):
                                rhs = xc[:, 4 * gi + ar:4 * gi + ar + 4,
                                         ac:ac + 2 * (W // 2):2]
                                nc.tensor.matmul(ps,
                                                 lhsT=wt[:, ar, 1 - ac:3 - ac, :],
                                                 rhs=rhs,
                                                 start=(k == 0), stop=(k == 5))
                                k += 1
                        ys = yout.tile([2 * NF, 4, W // 2], bf16)
                        nc.scalar.activation(ys, ps,
                                             mybir.ActivationFunctionType.Identity,
                                             bias=cb2_sb, scale=wstd2_sb)
                        nc.sync.dma_start(
                            out=yt[:, g * 2 * W:(g + 1) * 2 * W],
                            in_=ys)

    nc.compile()
    return nc


def _get(repeats=1):
    if repeats not in _CACHE:
        _CACHE[repeats] = _build(repeats)
    return _CACHE[repeats]


def _host_pack_x(x_b):
    """[H, W, CIN] fp32 -> padded bf16 [CIN, H+2, W+2]."""
    xb = np.zeros((CIN, H + 2, W + 2), dtype=ml_dtypes.bfloat16)
    xb[:, 1:H + 1, 1:W + 1] = np.ascontiguousarray(
        x_b.transpose(2, 0, 1)).astype(ml_dtypes.bfloat16)
    return xb


def _in_maps(x, w, fce_kernel, fce_bias, conv_kernel, conv_bias):
    JW = WDIM // 128
    fce_k = np.asarray(fce_kernel, np.float32)
    # fce_k row index = j*128 + p  ->  [p, j, c], scales baked in
    fkc = fce_k.reshape(JW, 128, CIN).transpose(1, 0, 2) * np.float32(
        FCE_SCALE * CONV_SCALE)
    cb = np.asarray(conv_bias, np.float32)
    ckb = np.empty((CIN, KK * KK * NF + 1), np.float32)
    # ck [ky, kx, c, n] -> [c, (ky kx n)]
    ckb[:, :KK * KK * NF] = np.asarray(conv_kernel, np.float32).transpose(
        2, 0, 1, 3).reshape(CIN, KK * KK * NF)
    ckb[:, KK * KK * NF] = np.asarray(fce_bias, np.float32) * np.float32(
        CONV_SCALE)
    maps = []
    for b in range(B):
        wvb = np.asarray(w[b], np.float32).reshape(JW, 128).T  # [p, j]
        fkw = np.empty((128, JW * (CIN + 1) + 1), np.float32)
        for j in range(JW):
            fkw[:, j * (CIN + 1):j * (CIN + 1) + CIN] = fkc[:, j, :]
            fkw[:, j * (CIN + 1) + CIN] = wvb[:, j]
        fkw[:, JW * (CIN + 1)] = np.concatenate([cb, cb])
        maps.append({
            "xb": _host_pack_x(np.asarray(x[b], np.float32)),
            "fkw": fkw,
            "ckb": ckb,
        })
    return maps


def kernel(x, w, fce_kernel, fce_bias, conv_kernel, conv_bias):
    from concourse.bass_utils import run_bass_kernel_spmd

    nc = _get()
    in_maps = _in_maps(x, w, fce_kernel, fce_bias, conv_kernel, conv_bias)
    res = run_bass_kernel_spmd(nc, in_maps, core_ids=list(range(NCORES)))
    out = np.empty((B, H, W, NF), np.float32)
    for b in range(B):
        yt = np.asarray(res.results[b]["yt"])         # [128, H*W/2] bf16
        # yt[s*NF+n, r*(W/2)+j] = y[r, 2j+s, n]
        y = yt.reshape(2, NF, H, W // 2).transpose(2, 3, 0, 1)
        out[b] = y.reshape(H, W, NF).astype(np.float32)
    return out


# revision 17
# speedup vs baseline: 2.4693x; 1.0249x over previous
"""ConvMod kernel for Trainium2 (8 NeuronCores, batch-parallel).

Per-sample modulated 3x3 grouped conv:
  style = w @ (fce_kernel*fce_scale) + fce_bias                [B, CIN]
  wp    = conv_kernel * conv_scale * style                     [B,3,3,CIN,NF]
  wpp   = wp * rsqrt(sum(wp^2, (ky,kx,cin)) + 1e-8)            demodulated
  out   = conv2d_same(x, wpp per-sample) + conv_bias           [B,H,W,NF]

Sharding: batch B=8 across 8 cores (1 sample/core), params replicated.

Device conv scheme (column-parity packing, bf16):
  - x is padded+cast to bf16 on host: xb[c, t, u] = x[t-1, u-1, c]
    ([CIN, H+2, W+2], zeros outside).
  - SBUF tile xc [128, 34, 258]: partitions 0-63 = chunk window of xb,
    partitions 64-127 = same window shifted left 2 columns.
  - One matmul computes BOTH column parities: out partition m=(s,n) with
    s = output-column parity, lhsT [128, 2, NF]; rhs reads columns with
    stride 2.  Slot (band, s) of matmul (ar, ac) carries conv tap
    ky=ar, kx=ac+2*band-s (weights zeroed for kx outside 0..2).
  - 6 matmuls (ar in 0..2, ac in 0..1) per 4-output-row strip accumulate
    the full 3x3x64 contraction into PSUM [128, 4, 128].
  - ACT evacuates PSUM -> bf16 SBUF with conv_bias, DMA to yt
    [128, H*W/2] (parity-interleaved); host de-interleaves.
"""

import numpy as np
import ml_dtypes

B, H, W, CIN = 8, 256, 256, 64
WDIM, NF, KK = 512, 64, 3
NCORES = 8
CR = 32             # output rows per chunk
NCH = H // CR       # chunks
SPC = CR // 4       # 4-row strips per chunk
FCE_SCALE = float(np.sqrt(1.0 / WDIM))
CONV_SCALE = float(np.sqrt(1.0 / 0.6 / (KK * KK * CIN)))

_CACHE = {}


def _build(repeats=1):
    import concourse.bass as bass_mod
    import concourse.mybir as mybir
    import concourse.tile as tile
    from concourse import bacc

    f32 = mybir.dt.float32
    bf16 = mybir.dt.bfloat16
    nc = bacc.Bacc("TRN2", target_bir_lowering=False, debug=False,
                   num_devices=NCORES)

    xbd = nc.dram_tensor("xb", [CIN, H + 2, W + 2], bf16,
                         kind="ExternalInput").ap()
    # host-packed params (fewer DMAs, scales pre-applied):
    # fkw [128, 4*65+1]: per j: 64 cols scaled fce_k + 1 col wv; last col
    # is conv_bias replicated for both parities.
    fkw_d = nc.dram_tensor("fkw", [128, (WDIM // 128) * (CIN + 1) + 1], f32,
                           kind="ExternalInput").ap()
    # ckb [64, 577]: conv_kernel as [c, (ky kx n)] + scaled fce_bias col.
    ckb_d = nc.dram_tensor("ckb", [CIN, KK * KK * NF + 1], f32,
                           kind="ExternalInput").ap()
    yt = nc.dram_tensor("yt", [2 * NF, H * W // 2], bf16,
                        kind="ExternalOutput").ap()

    NT = KK * KK  # 9 taps
    with tile.TileContext(nc) as tc:
        with (
            tc.tile_pool(name="const", bufs=1) as const,
            tc.tile_pool(name="prep", bufs=1) as prep,
            tc.tile_pool(name="pps", bufs=1, space="PSUM") as pps,
            tc.tile_pool(name="xin", bufs=4) as xin,
            tc.tile_pool(name="yout", bufs=6) as yout,
            tc.tile_pool(name="acc", bufs=4, space="PSUM") as accp,
            tc.tile_pool(name="warm", bufs=1, space="PSUM") as warmp,
        ):
            # ---- weight prep (tiny, fp32) ----
            JW = WDIM // 128
            ckb_sb = prep.tile([CIN, NT * NF + 1], f32)
            nc.sync.dma_start(out=ckb_sb, in_=ckb_d)
            fkw_sb = prep.tile([128, JW * (CIN + 1) + 1], f32)
            nc.sync.dma_start(out=fkw_sb, in_=fkw_d)
            cb2_sb = fkw_sb[:, JW * (CIN + 1):JW * (CIN + 1) + 1]

            ones_k = const.tile([CIN, 1], f32)
            nc.vector.memset(ones_k, 1.0)
            one1 = const.tile([1, 1], f32)
            nc.vector.memset(one1, 1.0)
            eps_sb = prep.tile([1, 1], f32)
            nc.vector.memset(eps_sb, 1e-8)
            # PE warmup: keep the tensor engine continuously busy through
            # the prep phase so the p-state ramp is at full speed (and
            # stays there) when the first conv matmuls dispatch.
            warm_sb = const.tile([128, 4 * (W // 2)], bf16)
            nc.vector.memset(warm_sb, 1.0)
            warm_ps = warmp.tile([2 * NF, 4, W // 2], f32)
            # Style-modulated (UN-demodulated) weights, parity-packed; the
            # demod rsqrt is applied per output channel at evacuation time
            # via the ACT scale operand.  wt[:, ar, jj, n]: the matmul for
            # (ar, ac) uses lhsT = wt[:, ar, 1-ac : 3-ac, :] so slot
            # (band, s) at jj = 1-ac+s holds tap ky=ar, kx=ac+2*band-s:
            #   band0 (parts 0-63):   jj0=kx1, jj1=kx0, jj2=zero
            #   band1 (parts 64-127): jj0=zero, jj1=kx2, jj2=kx1
            wt = const.tile([2 * CIN, KK, KK, NF], bf16)
            for ar in range(KK):
                nc.vector.memset(wt[0:CIN, ar, 2, :], 0.0)
                nc.vector.memset(wt[CIN:2 * CIN, ar, 0, :], 0.0)

            # style = w @ (fce_k * fce_scale * conv_scale)  (scales host-baked)
            style_ps = pps.tile([CIN, 1], f32)
            for j in range(JW):
                base = j * (CIN + 1)
                nc.tensor.matmul(style_ps, lhsT=fkw_sb[:, base:base + CIN],
                                 rhs=fkw_sb[:, base + CIN:base + CIN + 1],
                                 start=(j == 0), stop=(j == JW - 1))
            for _w in range(14):
                nc.tensor.matmul(warm_ps, lhsT=warm_sb[:, 0:128],
                                 rhs=warm_sb, start=True, stop=True)
            stylec = prep.tile([CIN, 1], f32)
            nc.vector.tensor_scalar_add(stylec, style_ps,
                                        ckb_sb[:, NT * NF:NT * NF + 1])

            # wp[c, t, n] = ck * stylec[c], then cast taps into wt slots
            wp = prep.tile([CIN, NT, NF], f32)
            nc.vector.tensor_scalar_mul(wp.rearrange("c t n -> c (t n)"),
                                        ckb_sb[:, 0:NT * NF],
                                        stylec)
            wp3 = wp.rearrange("c (t k) n -> c t k n", k=KK)
            nc.vector.tensor_copy(wt[0:CIN, :, 0, :], wp3[:, :, 1, :])
            nc.vector.tensor_copy(wt[0:CIN, :, 1, :], wp3[:, :, 0, :])
            nc.vector.tensor_copy(wt[CIN:2 * CIN, :, 1, :], wp3[:, :, 2, :])
            nc.vector.tensor_copy(wt[CIN:2 * CIN, :, 2, :], wp3[:, :, 1, :])

            # chunk 0 load + band copies hoisted ahead of the demod chain:
            # the conv only needs wt + chunk0 bands; demod (wstd2) is first
            # consumed by the evacuation of strip 0, ~1.3us later.
            xc0 = xin.tile([2 * CIN, CR + 2, W + 2], bf16)
            nc.sync.dma_start(out=xc0[0:CIN, 0:8, :], in_=xbd[:, 0:8, :])
            nc.sync.dma_start(out=xc0[0:CIN, 8:20, :], in_=xbd[:, 8:20, :])
            nc.sync.dma_start(out=xc0[0:CIN, 20:CR + 2, :],
                              in_=xbd[:, 20:CR + 2, :])
            for (a, b) in ((0, 6), (6, 14), (14, 24), (24, CR + 2)):
                nc.vector.tensor_copy(xc0[CIN:2 * CIN, a:b, 0:W],
                                      xc0[0:CIN, a:b, 2:W + 2])

            # demod: wstd2[(s n), 1] = rsqrt(sum(wp^2) + 1e-8) per channel,
            # replicated for both parities; consumed by the evac ACT scale.
            sq = prep.tile([CIN, NT, NF], f32)
            nc.vector.tensor_mul(sq.rearrange("c t n -> c (t n)"),
                                 wp.rearrange("c t n -> c (t n)"),
                                 wp.rearrange("c t n -> c (t n)"))
            tap_acc = prep.tile([CIN, NF], f32)
            nc.vector.tensor_reduce(tap_acc, sq.rearrange("c t n -> c n t"),
                                    mybir.AxisListType.X, mybir.AluOpType.add)
            ssum_ps = pps.tile([1, NF], f32)
            nc.tensor.matmul(ssum_ps, lhsT=ones_k, rhs=tap_acc,
                             start=True, stop=True)
            sroot = prep.tile([1, NF], f32)
            nc.scalar.activation(sroot, ssum_ps,
                                 mybir.ActivationFunctionType.Sqrt,
                                 bias=eps_sb, scale=1.0)
            wstd = prep.tile([1, NF], f32)
            nc.vector.reciprocal(wstd, sroot)
            wstd2_ps = pps.tile([NF, 1], f32)
            nc.tensor.matmul(wstd2_ps, lhsT=wstd, rhs=one1,
                             start=True, stop=True)
            wstd2_sb = const.tile([2 * NF, 1], f32)
            nc.vector.tensor_copy(wstd2_sb[0:NF, :], wstd2_ps)
            nc.vector.tensor_copy(wstd2_sb[NF:2 * NF, :], wstd2_ps)

            # ---- main conv loop ----
            for _ in range(repeats):
                for ci in range(NCH):
                    L = ci * CR
                    if ci == 0 and xc0 is not None:
                        xc = xc0
                        xc0 = None
                    else:
                        xc = xin.tile([2 * CIN, CR + 2, W + 2], bf16)
                        nc.sync.dma_start(out=xc[0:CIN, :, :],
                                          in_=xbd[:, L:L + CR + 2, :])
                        # band B: same rows shifted left two columns; split
                        # so early strips start before the whole copy lands
                        for (a, b) in ((0, 12), (12, 24), (24, CR + 2)):
                            nc.vector.tensor_copy(
                                xc[CIN:2 * CIN, a:b, 0:W],
                                xc[0:CIN, a:b, 2:W + 2])
                    for gi in range(SPC):
                        g = ci * SPC + gi
                        ps = accp.tile([2 * NF, 4, W // 2], f32)
                        k = 0
                        for ar in range(KK):
                            for ac in range(2):
                                rhs = xc[:, 4 * gi + ar:4 * gi + ar + 4,
                                         ac:ac + 2 * (W // 2):2]
                                nc.tensor.matmul(ps,
                                                 lhsT=wt[:, ar, 1 - ac:3 - ac, :],
                                                 rhs=rhs,
                                                 start=(k == 0), stop=(k == 5))
                                k += 1
                        ys = yout.tile([2 * NF, 4, W // 2], bf16)
                        nc.scalar.activation(ys, ps,
                                             mybir.ActivationFunctionType.Identity,
                                             bias=cb2_sb, scale=wstd2_sb)
                        nc.sync.dma_start(
                            out=yt[:, g * 2 * W:(g + 1) * 2 * W],
                            in_=ys)

    nc.compile()
    return nc


def _get(repeats=1):
    if repeats not in _CACHE:
        _CACHE[repeats] = _build(repeats)
    return _CACHE[repeats]


def _host_pack_x(x_b):
    """[H, W, CIN] fp32 -> padded bf16 [CIN, H+2, W+2]."""
    xb = np.zeros((CIN, H + 2, W + 2), dtype=ml_dtypes.bfloat16)
    xb[:, 1:H + 1, 1:W + 1] = np.ascontiguousarray(
        x_b.transpose(2, 0, 1)).astype(ml_dtypes.bfloat16)
    return xb


def _in_maps(x, w, fce_kernel, fce_bias, conv_kernel, conv_bias):
    JW = WDIM // 128
    fce_k = np.asarray(fce_kernel, np.float32)
    # fce_k row index = j*128 + p  ->  [p, j, c], scales baked in
    fkc = fce_k.reshape(JW, 128, CIN).transpose(1, 0, 2) * np.float32(
        FCE_SCALE * CONV_SCALE)
    cb = np.asarray(conv_bias, np.float32)
    ckb = np.empty((CIN, KK * KK * NF + 1), np.float32)
    # ck [ky, kx, c, n] -> [c, (ky kx n)]
    ckb[:, :KK * KK * NF] = np.asarray(conv_kernel, np.float32).transpose(
        2, 0, 1, 3).reshape(CIN, KK * KK * NF)
    ckb[:, KK * KK * NF] = np.asarray(fce_bias, np.float32) * np.float32(
        CONV_SCALE)
    maps = []
    for b in range(B):
        wvb = np.asarray(w[b], np.float32).reshape(JW, 128).T  # [p, j]
        fkw = np.empty((128, JW * (CIN + 1) + 1), np.float32)
        for j in range(JW):
            fkw[:, j * (CIN + 1):j * (CIN + 1) + CIN] = fkc[:, j, :]
            fkw[:, j * (CIN + 1) + CIN] = wvb[:, j]
        fkw[:, JW * (CIN + 1)] = np.concatenate([cb, cb])
        maps.append({
            "xb": _host_pack_x(np.asarray(x[b], np.float32)),
            "fkw": fkw,
            "ckb": ckb,
        })
    return maps


def kernel(x, w, fce_kernel, fce_bias, conv_kernel, conv_bias):
    from concourse.bass_utils import run_bass_kernel_spmd

    nc = _get()
    in_maps = _in_maps(x, w, fce_kernel, fce_bias, conv_kernel, conv_bias)
    res = run_bass_kernel_spmd(nc, in_maps, core_ids=list(range(NCORES)))
    out = np.empty((B, H, W, NF), np.float32)
    for b in range(B):
        yt = np.asarray(res.results[b]["yt"])         # [128, H*W/2] bf16
        # yt[s*NF+n, r*(W/2)+j] = y[r, 2j+s, n]
        y = yt.reshape(2, NF, H, W // 2).transpose(2, 3, 0, 1)
        out[b] = y.reshape(H, W, NF).astype(np.float32)
    return out
